# revision 31
# baseline (speedup 1.0000x reference)
"""Trainium2 Bass kernel for nn_AdvancedIFTransformerClassifier.

Self-contained: takes FULL inputs (as from setup_inputs()), shards batch
B=512 across 8 NeuronCores (64 samples each, pure data parallel), runs one
compiled Bass/Tile program per core, gathers [512, 100] output.

Design notes (v2):
- All GEMMs in fp32r (FP22 multiply, fp32 accumulate) at full PE rate.
- LayerNorms are folded into the adjacent GEMMs: the per-channel gain g and
  the mean subtraction are folded into the weights on the host
  (W~ = g*W - colsum(g*W)/E gives h @ W~ == ((h - mean)*g) @ W exactly);
  the per-token rstd is applied as a per-partition activation scale
  (token-major) or one vector multiply (channel-major). Stats (mean, meansq)
  are computed on the PE via a 1/E-column contraction.
- Attention runs on 16 uniform chunks of 4 samples (100 tokens). The DFT
  real/imag matrices are stacked along the output dim ([100, 104]) so one
  matmul produces both components; the real-part cross-add of conj(q)*k is
  done by a [104, 52] summing matrix on the PE; the inverse DFT is fused
  with the transpose back to channel-major (o_cm = vs^T @ Dstk).
- FFN f2 accumulates over th-pairs in PSUM (8-matmul groups) to halve the
  residual adds on the vector engine.
"""
import sys
import types
import numpy as np
from contextlib import ExitStack


def _install_ntff_hook():
    try:
        import antenv.axon_hooks  # noqa: F401
        return
    except ImportError:
        pass
    try:
        from trn_agent_boot.trn_boot import _ntff_profile_via_ctypes
        hook = _ntff_profile_via_ctypes('/opt/axon/libaxon_pjrt.so')
    except Exception:
        hook = None
    mod = types.ModuleType('antenv.axon_hooks')
    mod._hook = hook
    mod.get_axon_ntff_profile_hook = lambda: mod._hook
    mod.set_axon_ntff_profile_hook = lambda h: setattr(mod, '_hook', h)
    sys.modules['antenv.axon_hooks'] = mod


_install_ntff_hook()

import concourse.bass as bass  # noqa: E402
import concourse.tile as tile  # noqa: E402
from concourse import bacc, mybir  # noqa: E402
from concourse.bass_utils import run_bass_kernel_spmd  # noqa: E402

FP32 = mybir.dt.float32
FP32R = mybir.dt.float32r
AF = mybir.ActivationFunctionType
ALU = mybir.AluOpType
AX = mybir.AxisListType

# ---- problem dims (hardcoded) ----
B, T, DIN = 512, 96, 64
H, PATCH, L, HEADS, NCLS = 256, 2, 4, 8, 100
E = 3 * H                    # 768
T2 = T // PATCH              # 48
F = T2 // 2 + 1              # 25 (transformer seq len)
FQ = F // 2 + 1              # 13
DH = E // HEADS              # 96
EPS = 1e-5
NCORES = 8
BL = B // NCORES             # 64 samples/core
TOKS = BL * F                # 1600 transformer tokens/core
NCH = E // 128               # 6 channel chunks
WZ = 1800                    # zb per-chunk width (>= 1+BL*28 = 1793)
WH = 1600                    # h per-chunk width
PADW = 1 + BL * 28           # 1793
NCHUNK = 16                  # attention chunks (4 samples each)
CTOK = 100                   # tokens per chunk
CFR = 104                    # stacked freq rows (52 real + 52 imag)

# token tiles for c-major GEMMs (uniform 400 to avoid ldweights-bound tails)
TT4 = [(0, 400), (400, 400), (800, 400), (1200, 400)]
# b-aligned tiles (for conv / stage0): (sample_off, nsamples), n = nb*25 = 400
BT4 = [(0, 16), (16, 16), (32, 16), (48, 16)]
# qkv column groups (keep streams >= 384 so ldweights stays hidden)
QKVG = [(0, 512), (512, 512), (1024, 512), (1536, 384), (1920, 384)]


def _np_consts():
    t2 = np.arange(T2)[:, None]
    f = np.arange(F)[None, :]
    C48 = np.cos(2 * np.pi * t2 * f / T2)                      # [48, 25]
    c48bd = np.zeros((96, 50))
    c48bd[0:48, 0:25] = C48
    c48bd[48:96, 25:50] = C48

    tt = np.arange(F)[:, None]
    fq = np.arange(FQ)[None, :]
    C25r = np.cos(2 * np.pi * tt * fq / F)                     # [25, 13]
    C25i = -np.sin(2 * np.pi * tt * fq / F)
    a = np.full(FQ, 2.0 / F)
    a[0] = 1.0 / F
    Dr = a[:, None] * np.cos(2 * np.pi * np.arange(FQ)[:, None] * np.arange(F)[None, :] / F)
    Di = -a[:, None] * np.sin(2 * np.pi * np.arange(FQ)[:, None] * np.arange(F)[None, :] / F)

    sc = DH ** -0.25

    def blockdiag(m, nb):
        r, c = m.shape
        out = np.zeros((r * nb, c * nb))
        for i in range(nb):
            out[i * r:(i + 1) * r, i * c:(i + 1) * c] = m
        return out

    cst = {}
    # stacked DFT matrices for 4-sample chunks: [100, 104] (real | imag)
    cqk = np.zeros((CTOK, CFR))
    cqk[:, 0:52] = blockdiag(C25r * sc, 4)
    cqk[:, 52:104] = blockdiag(C25i * sc, 4)
    cv = np.zeros((CTOK, CFR))
    cv[:, 0:52] = blockdiag(C25r, 4)
    cv[:, 52:104] = blockdiag(C25i, 4)
    dstk = np.zeros((CFR, CTOK))
    dstk[0:52, :] = blockdiag(Dr, 4)
    dstk[52:104, :] = blockdiag(Di, 4)
    ssum = np.zeros((CFR, 52))
    ssum[0:52, :] = np.eye(52)
    ssum[52:104, :] = np.eye(52)
    cst["cqk_stk"] = cqk
    cst["cv_stk"] = cv
    cst["d_stk"] = dstk
    cst["ssum"] = ssum
    cst["c48bd"] = c48bd
    cst["eye"] = np.eye(128)
    cst["inv_e"] = np.full((128, 1), 1.0 / E)
    cst["ones_row"] = np.ones((1, 128))
    return {k: v.astype(np.float32) for k, v in cst.items()}


def _r22(x):
    """Round to nearest FP22 so the PE's fp32r truncation becomes exact."""
    u = (np.ascontiguousarray(x, np.float32).view(np.uint32) + (1 << 9)) & np.uint32(0xFFFFFC00)
    return u.view(np.float32)


def _pp(v, nch):
    """[C] -> [128, nch] per-partition layout (col j = chunk j)."""
    return np.ascontiguousarray(v.reshape(nch, 128).T).astype(np.float32)


def _center(w, g):
    """Fold LN gain + mean-subtraction into a weight matrix.

    w: [C, O], g: [C].  Returns W~ with h @ W~ == ((h - mean_c(h)) * g) @ w.
    """
    wg = w * g[:, None]
    return wg - wg.sum(axis=0, keepdims=True) / w.shape[0]


def _prep(inputs):
    """Host-side prep: returns (shared_map, per-core x list)."""
    g = {k: np.asarray(v, dtype=np.float32) for k, v in inputs.items()}
    cst = _np_consts()
    sh = dict(cst)

    sh["w_in"] = g["W_in"]                                      # [64, 256]
    sh["pe_eff"] = np.ascontiguousarray(
        (g["pe"][0, :T, :] + g["b_in"][None, :]).T)             # [256, 96]
    sh["w_shape"] = g["W_shape"]                                # [256, 256]
    sh["b_shape_pp"] = _pp(g["b_shape"], 2)
    sh["w_patch"] = g["W_patch"]                                # [512, 256]
    sh["b_patch_row"] = g["b_patch"][None, :]                   # [1, 256]
    taps = []
    for wname in ("conv_w1", "conv_w2", "conv_w4"):
        w = g[wname]                                            # [O, I, k]
        for kk in range(w.shape[2]):
            taps.append(np.ascontiguousarray(w[:, :, kk].T))
    sh["convw"] = np.stack(taps)                                # [7, 256, 256]
    sh["conv_b_pp"] = _pp(np.concatenate([g["conv_b1"], g["conv_b2"], g["conv_b4"]]), 6)

    # ---- LN-folded transformer weights ----
    # qkv: fold ln1 gain + centering; ln1_b must be zero (token-major bias
    # has no cheap slot; setup_inputs() keeps it zero).
    assert np.abs(g["ln1_b"]).max() == 0.0, "nonzero ln1_b not supported"
    sh["wqkv"] = np.stack([_center(g["Wqkv"][i], g["ln1_g"][i]) for i in range(L)])
    sh["wo"] = g["Wo"]                                          # [4, 768, 768]
    sh["bo_pp"] = np.concatenate([_pp(g["bo"][i], 6) for i in range(L)], 1)
    # f1: fold ln2 gain + centering; ln2_b folds into the gelu bias.
    sh["wf1"] = np.stack([_center(g["Wf1"][i], g["ln2_g"][i]) for i in range(L)])
    bf1_eff = g["bf1"] + np.einsum("lc,lco->lo", g["ln2_b"], g["Wf1"])
    sh["bf1_pp"] = np.concatenate([_pp(bf1_eff[i], 24) for i in range(L)], 1)
    sh["wf2"] = g["Wf2"]                                        # [4, 3072, 768]
    sh["bf2_pp"] = np.concatenate([_pp(g["bf2"][i], 6) for i in range(L)], 1)
    # ssm conv taps [3, 768, 768] (cin, cout)
    sh["ssmw"] = np.stack([np.ascontiguousarray(g["ssm_w"][:, :, kk].T) for kk in range(3)])
    sh["ssmb_pp"] = _pp(g["ssm_b"], 6)
    # head: fold final-LN gain + centering + mean-over-f (/F); ssm_bn folds
    # into the output bias (sum over F freqs of cbn/F = bn @ W_out).
    sh["wout"] = _center(g["W_out"] / F, g["ssm_g"])            # [768, 100]
    sh["bout_pp"] = (g["b_out"] + g["ssm_bn"] @ g["W_out"])[:, None]  # [100, 1]

    x = g["x"]                                                  # [512, 96, 64]
    xs = []
    for c in range(NCORES):
        xc = x[c * BL:(c + 1) * BL]                             # [64, 96, 64]
        xs.append(np.ascontiguousarray(xc.transpose(2, 0, 1).reshape(DIN, BL * T)))
    sh = {k: _r22(np.ascontiguousarray(v, dtype=np.float32)) for k, v in sh.items()}
    xs = [_r22(v) for v in xs]
    return sh, xs


# ---------------------------------------------------------------------------
def _build():
    nc = bacc.Bacc("TRN2", target_bir_lowering=False, debug=False, num_devices=NCORES)

    def din(name, shape):
        return nc.dram_tensor(name, list(shape), FP32, kind="ExternalInput")

    d = {}
    d["xcm"] = din("xcm", [DIN, BL * T])
    d["w_in"] = din("w_in", [DIN, H])
    d["pe_eff"] = din("pe_eff", [H, T])
    d["w_shape"] = din("w_shape", [H, H])
    d["b_shape_pp"] = din("b_shape_pp", [128, 2])
    d["w_patch"] = din("w_patch", [2 * H, H])
    d["b_patch_row"] = din("b_patch_row", [1, H])
    d["convw"] = din("convw", [7, H, H])
    d["conv_b_pp"] = din("conv_b_pp", [128, 6])
    d["wqkv"] = din("wqkv", [L, E, 3 * E])
    d["wo"] = din("wo", [L, E, E])
    d["bo_pp"] = din("bo_pp", [128, 6 * L])
    d["wf1"] = din("wf1", [L, E, 4 * E])
    d["bf1_pp"] = din("bf1_pp", [128, 24 * L])
    d["wf2"] = din("wf2", [L, 4 * E, E])
    d["bf2_pp"] = din("bf2_pp", [128, 6 * L])
    d["ssmw"] = din("ssmw", [3, E, E])
    d["ssmb_pp"] = din("ssmb_pp", [128, 6])
    d["wout"] = din("wout", [E, NCLS])
    d["bout_pp"] = din("bout_pp", [NCLS, 1])
    for nm, shp in (("c48bd", [96, 50]), ("eye", [128, 128]),
                    ("inv_e", [128, 1]), ("ones_row", [1, 128]),
                    ("cqk_stk", [CTOK, CFR]), ("cv_stk", [CTOK, CFR]),
                    ("d_stk", [CFR, CTOK]), ("ssum", [CFR, 52])):
        d[nm] = din(nm, shp)
    out_d = nc.dram_tensor("out", [BL, NCLS], FP32, kind="ExternalOutput")

    with tile.TileContext(nc) as tc, ExitStack() as ctx:
        _program(nc, tc, ctx, d, out_d)
    nc.compile()
    return nc


def _program(nc, tc, ctx, d, out_d):
    V, S = nc.vector, nc.scalar

    ps = ctx.enter_context(tc.tile_pool(name="ps", bufs=1, space="PSUM"))
    cst = ctx.enter_context(tc.tile_pool(name="cst", bufs=1))
    per = ctx.enter_context(tc.tile_pool(name="per", bufs=1))
    tmp = ctx.enter_context(tc.tile_pool(name="tmp", bufs=1))

    def pst(shape, tag, bufs):
        return ps.tile(shape, FP32, tag=tag, bufs=bufs, name=f"ps_{tag}")

    # ---- persistent activation buffers ----
    h = per.tile([128, NCH * WH], FP32R, tag="h", name="h")        # h chunks, stride WH
    zb = per.tile([128, NCH * WZ], FP32R, tag="zb", name="zb")     # z / o / h_pad

    # ---- constants in SBUF (stage0-critical loads emitted now; the rest
    # deferred until after stage0 emission so the first matmuls start early) ----
    deferred_loads = []

    def cload(name, shape, dt=FP32R, defer=False):
        t = cst.tile(list(shape), dt, tag=name, name=name)
        ap = d[name][:, :]
        ap = ap.bitcast(dt) if dt == FP32R else ap
        if defer:
            deferred_loads.append((t[0:shape[0], 0:shape[1]], ap))
        else:
            nc.sync.dma_start(t[0:shape[0], 0:shape[1]], ap)
        return t

    ones_row = cload("ones_row", [1, 128])
    c48bd = cload("c48bd", [96, 50])
    b_patch_row = cload("b_patch_row", [1, H])
    eye = cload("eye", [128, 128], FP32, defer=True)
    inv_e = cload("inv_e", [128, 1], defer=True)
    cqk_stk = cload("cqk_stk", [CTOK, CFR], defer=True)
    cv_stk = cload("cv_stk", [CTOK, CFR], defer=True)
    d_stk = cload("d_stk", [CFR, CTOK], defer=True)
    ssum = cload("ssum", [CFR, 52], defer=True)
    pp = {}
    for nm in ("b_shape_pp", "conv_b_pp", "bo_pp", "bf1_pp", "bf2_pp", "ssmb_pp"):
        shp = list(d[nm].shape)
        pp[nm] = cst.tile(shp, FP32, tag=nm, name=nm)
        if nm in ("b_shape_pp", "conv_b_pp"):
            nc.sync.dma_start(pp[nm][:, :], d[nm][:, :])
        else:
            deferred_loads.append((pp[nm][:, :], d[nm][:, :]))
    bout_pp = cst.tile([NCLS, 1], FP32, tag="bout_pp", name="bout_pp")
    deferred_loads.append((bout_pp[0:NCLS, :], d["bout_pp"][:, :]))

    def ttile(shape, tag, bufs, dt=FP32):
        return tmp.tile(list(shape), dt, tag=tag, bufs=bufs, name=f"t_{tag}")

    # =======================================================================
    # Stage 0 (processed in 4-sample blocks)
    # =======================================================================
    with tc.tile_pool(name="s0f", bufs=1) as s0f:
      fp = [s0f.tile([128, WZ], FP32R, tag="fpad", bufs=2, name="fpad") for _ in range(2)]
      cw = s0f.tile([128, 14 * H], FP32R, tag="cw", name="cw")
      with tc.tile_pool(name="s0a", bufs=1) as s0:
        wi = s0.tile([64, H], FP32R, tag="wi", name="wi")
        nc.sync.dma_start(wi[0:64, :], d["w_in"][:, :].bitcast(FP32R))
        nc.sync.dma_start(
            cw[:, :].rearrange("p (k c o) -> p k c o", k=7, c=2),
            d["convw"][:, :, :].rearrange("k (c p) o -> p k c o", p=128).bitcast(FP32R))
        for ci in range(2):
            V.memset(fp[ci][:, :].bitcast(FP32), 0.0)

        def chunked_load(pool, name, dsrc, width, dt=FP32R):
            nch_ = dsrc.shape[0] // 128
            t = pool.tile([128, nch_ * width], dt, tag=name, name=name)
            nc.sync.dma_start(
                t[:, :].rearrange("p (c o) -> p c o", o=width),
                dsrc.rearrange("(c p) o -> p c o", p=128).bitcast(dt) if dt == FP32R
                else dsrc.rearrange("(c p) o -> p c o", p=128))
            return t

        pe = chunked_load(s0, "pe", d["pe_eff"][:, :], T, FP32)
        wsh = chunked_load(s0, "wsh", d["w_shape"][:, :], H)
        wpa = chunked_load(s0, "wpa", d["w_patch"][:, :], H)

        for j in range(16):                        # blocks of 4 samples
            b0 = 4 * j
            off = b0 * T
            n = 4 * T                              # 384
            xt = s0.tile([64, 384], FP32R, tag="xin", bufs=3, name="xt")
            nc.sync.dma_start(xt[0:64, 0:n], d["xcm"][:, off:off + n].bitcast(FP32R))
            h1b = []
            for co in range(2):
                hb = s0.tile([128, 384], FP32R, tag="h1b", bufs=4, name="h1b")
                p = pst([128, 512], "rot", 2)
                nc.tensor.matmul(p[:, 0:n], wi[0:64, co * 128:(co + 1) * 128],
                                 xt[0:64, 0:n], start=True, stop=True)
                pe_b = pe[:, co * T:(co + 1) * T].unsqueeze(1).to_broadcast([128, 4, T])
                V.tensor_tensor(hb[:, 0:n].rearrange("p (b t) -> p b t", t=T),
                                p[:, 0:n].rearrange("p (b t) -> p b t", t=T),
                                pe_b, ALU.add)
                h1b.append(hb)
            sfb = []
            for ci in range(2):
                sft = s0.tile([128, 384], FP32R, tag="sf", bufs=4, name="sf")
                V.tensor_tensor(sft[:, 1:n], h1b[ci][:, 1:n],
                                h1b[ci][:, 0:n - 1], ALU.subtract)
                V.memset(sft[:, 0:n].rearrange("p (b t) -> p b t", t=T)[:, :, 0:1]
                         .bitcast(FP32), 0.0)
                sfb.append(sft)
            for co in range(2):
                p = pst([128, 512], "rot", 2)
                for ci in range(2):
                    nc.tensor.matmul(p[:, 0:n],
                                     wsh[:, ci * H + co * 128: ci * H + (co + 1) * 128],
                                     sfb[ci][:, 0:n], start=(ci == 0), stop=(ci == 1))
                V.affine_then_add(h1b[co][:, 0:n], p[:, 0:n], h1b[co][:, 0:n],
                                  1.0, pp["b_shape_pp"][:, co:co + 1])
            # patch (2 groups of 2 samples) + rfft48 fused with transpose
            for g2 in range(2):
                hp_ps = pst([96, 256], "rot", 2)
                nc.tensor.matmul(hp_ps[0:96, :], ones_row[0:1, 0:96], b_patch_row[:, :],
                                 start=True, stop=False)
                for cp in range(4):
                    p_half, ci = cp // 2, cp % 2
                    lct = s0.tile([128, 96], FP32R, tag="lct", bufs=6, name="lct")
                    S.activation(lct[:, :].rearrange("p (b t) -> p b t", t=T2),
                                 h1b[ci][:, g2 * 192:(g2 + 1) * 192].rearrange(
                                     "p (b t) -> p b t", t=T)[:, :, p_half:T:2].bitcast(FP32),
                                 AF.Copy)
                    nc.tensor.matmul(hp_ps[0:96, :], lct[:, :], wpa[:, cp * H:(cp + 1) * H],
                                     start=False, stop=(cp == 3))
                hp_t = s0.tile([96, 256], FP32R, tag="hpt", bufs=3, name="hpt")
                S.activation(hp_t[0:96, :], hp_ps[0:96, :], AF.Copy)
                # fused rfft + transpose: f_cm[ch, (b, f)] = hp^T @ c48bd
                bb0 = b0 + 2 * g2
                for ci in range(2):
                    fr_ps = pst([128, 64], "aux", 2)
                    nc.tensor.matmul(fr_ps[0:128, 0:50],
                                     hp_t[0:96, ci * 128:(ci + 1) * 128],
                                     c48bd[0:96, 0:50], start=True, stop=True)
                    V.tensor_copy(
                        fp[ci][:, 1 + bb0 * 28: 1 + (bb0 + 2) * 28].rearrange(
                            "p (b f) -> p b f", f=28)[:, :, 0:25],
                        fr_ps[:, 0:50].rearrange("p (b f) -> p b f", f=25))

      # --- multiscale convs -> h [768, 1600] ---
      with tc.tile_pool(name="s0b", bufs=1) as s0b:
        conv_taps = {0: [(0, 0)], 1: [(1, 0), (2, 1)],
                     2: [(3, -1), (4, 0), (5, 1), (6, 2)]}
        shifts = (-1, 0, 1, 2)
        for (boff, nb) in BT4:
            n = nb * 25
            crhs = {}
            for sh_ in shifts:
                for ci in range(2):
                    ct = s0b.tile([128, 512], FP32R, tag="crhs", bufs=8, name="crhs")
                    S.activation(ct[:, 0:n].rearrange("p (b f) -> p b f", f=25),
                                 fp[ci][:, 1 + sh_ + boff * 28:][0:128, 0:nb * 28]
                                 .rearrange("p (b f) -> p b f", f=28)[:, :, 0:25].bitcast(FP32),
                                 AF.Copy)
                    crhs[(sh_, ci)] = ct
            for co6 in range(NCH):
                m_idx, co_m = co6 // 2, co6 % 2
                taps = conv_taps[m_idx]
                p = pst([128, 512], "rot", 2)
                first = True
                for (tap, sh_) in taps:
                    for ci in range(2):
                        nc.tensor.matmul(p[:, 0:n],
                                         cw[:, (tap * 2 + ci) * H + co_m * 128:
                                            (tap * 2 + ci) * H + (co_m + 1) * 128],
                                         crhs[(sh_, ci)][:, 0:n], start=first,
                                         stop=(tap == taps[-1][0] and ci == 1))
                        first = False
                V.tensor_scalar(h[:, co6 * WH + boff * 25: co6 * WH + boff * 25 + n],
                                p[:, 0:n], pp["conv_b_pp"][:, co6:co6 + 1], None, ALU.add)

    for dst, src_ap in deferred_loads:
        nc.sync.dma_start(dst, src_ap)

    # =======================================================================
    # LayerNorm stats helpers
    # =======================================================================
    G = nc.gpsimd

    def emit_stats(dst_m, dst_q, src, stride, off, n):
        """PE-assisted stats for cols [off, off+n): mean -> dst_m row,
        mean-square -> dst_q row.  Squares run on the (idle) gpsimd engine
        so the scalar activation table never thrashes."""
        mp = pst([1, 512], "bi", 2)
        sp = pst([1, 512], "bi", 2)
        for ci in range(NCH):
            sl = src[:, ci * stride + off: ci * stride + off + n]
            nc.tensor.matmul(mp[0:1, 0:n], inv_e[:, :], sl,
                             start=(ci == 0), stop=(ci == NCH - 1))
            sq = ttile([128, 512], "u6", 2, FP32R)
            S.activation(sq[:, 0:n], sl.bitcast(FP32), AF.Square)
            nc.tensor.matmul(sp[0:1, 0:n], inv_e[:, :], sq[:, 0:n],
                             start=(ci == 0), stop=(ci == NCH - 1))
        V.tensor_copy(dst_m[0:1, off:off + n], mp[0:1, 0:n])
        V.tensor_copy(dst_q[0:1, off:off + n], sp[0:1, 0:n])

    def emit_rowmath(pool, rows_m, rows_q, rrow, off, n):
        """rstd row (f32r) for cols [off, off+n): 1/sqrt(msq - m^2 + eps)."""
        u = pool.tile([1, 512], FP32, tag="rm1", bufs=2, name="rm1")
        V.tensor_tensor(u[0:1, 0:n], rows_m[0:1, off:off + n],
                        rows_m[0:1, off:off + n], ALU.mult)
        V.tensor_tensor(u[0:1, 0:n], rows_q[0:1, off:off + n], u[0:1, 0:n],
                        ALU.subtract)
        V.tensor_scalar(u[0:1, 0:n], u[0:1, 0:n], EPS, None, ALU.add)
        lg = pool.tile([1, 512], FP32, tag="rm2", bufs=2, name="rm2")
        S.activation(lg[0:1, 0:n], u[0:1, 0:n], AF.Sqrt)
        with nc.allow_low_precision(reason="f32r rstd row"):
            V.reciprocal(rrow[0:1, off:off + n], lg[0:1, 0:n])

    # =======================================================================
    # Transformer layers
    # =======================================================================
    for l in range(L):
        with tc.tile_pool(name=f"at{l}", bufs=1) as ap_:
            def atile(shape, tag, bufs, dt=FP32):
                return ap_.tile(list(shape), dt, tag=tag, bufs=bufs, name=f"a_{tag}")

            # ---- qkv weights (DMA prefetch overlaps LN1 stats) ----
            wq = []
            for ci in range(NCH):
                w = ap_.tile([128, 3 * E], FP32R, tag=f"wq{ci}", bufs=1, name=f"wq{l}_{ci}")
                nc.sync.dma_start(w[:, :], d["wqkv"][l, ci * 128:(ci + 1) * 128, :].bitcast(FP32R))
                wq.append(w)

            # ---- LN1 stats -> rstd columns [100, 16] (tokens on partitions) ----
            rows1 = atile([33, TOKS], "rows1", 1)    # p0 = m, p32 = msq
            mqcol = atile([CTOK, 2 * NCHUNK], "mqcol", 1)
            rstdc = atile([CTOK, NCHUNK], "rstdc", 1)
            V.memset(rows1[0:33, :], 0.0)
            for ti, (off, n) in enumerate(TT4):
                emit_stats(rows1[0:1, :], rows1[32:33, :], h, WH, off, n)
                for c in range(4 * ti, 4 * ti + 4):
                    tp = pst([128, 128], "t", 2)
                    nc.tensor.transpose(tp[0:CTOK, 0:33],
                                        rows1[0:33, c * CTOK:(c + 1) * CTOK],
                                        eye[0:33, 0:33])
                    V.tensor_copy(mqcol[0:CTOK, 2 * c:2 * c + 1], tp[0:CTOK, 0:1])
                    V.tensor_copy(mqcol[0:CTOK, 2 * c + 1:2 * c + 2], tp[0:CTOK, 32:33])
                mc = mqcol[0:CTOK, 8 * ti:8 * ti + 8].rearrange(
                    "p (c k) -> p c k", k=2)[:, :, 0:1]
                qc = mqcol[0:CTOK, 8 * ti:8 * ti + 8].rearrange(
                    "p (c k) -> p c k", k=2)[:, :, 1:2]
                vc = atile([CTOK, 4], "vc", 2)
                V.tensor_tensor(vc[0:CTOK, 0:4].unsqueeze(2), mc, mc, ALU.mult)
                V.tensor_tensor(vc[0:CTOK, 0:4].unsqueeze(2), qc,
                                vc[0:CTOK, 0:4].unsqueeze(2), ALU.subtract)
                V.tensor_scalar(vc[0:CTOK, 0:4], vc[0:CTOK, 0:4], EPS, None, ALU.add)
                lc = atile([CTOK, 4], "lc", 2)
                S.activation(lc[0:CTOK, 0:4], vc[0:CTOK, 0:4], AF.Sqrt)
                V.reciprocal(rstdc[0:CTOK, 4 * ti:4 * ti + 4], lc[0:CTOK, 0:4])

            # ---- attention: software-pipelined over 16 chunks ----
            def emit_qkv(c):
                coff = c * CTOK
                qs = atile([CTOK, 3 * E], "qs", 3, FP32R)
                for (qo, qn) in QKVG:
                    p = pst([128, 512], "rot", 2)
                    for ci in range(NCH):
                        nc.tensor.matmul(p[0:CTOK, 0:qn],
                                         h[:, ci * WH + coff: ci * WH + coff + CTOK],
                                         wq[ci][:, qo:qo + qn],
                                         start=(ci == 0), stop=(ci == NCH - 1))
                    S.activation(qs[0:CTOK, qo:qo + qn], p[0:CTOK, 0:qn], AF.Copy,
                                 scale=rstdc[0:CTOK, c:c + 1])
                return qs

            def emit_mid_front(c, qs):
                # q/k DFT (stacked real|imag) + logits + softmax
                lp = atile([CFR, 8], "lp", 2, FP32R)
                for half in range(2):
                    qps = pst([128, 512], "aux", 2)
                    nc.tensor.matmul(qps[0:CFR, 0:384], cqk_stk[0:CTOK, 0:CFR],
                                     qs[0:CTOK, half * 384: half * 384 + 384],
                                     start=True, stop=True)
                    q_s = atile([CFR, 384], "q_s", 2)
                    S.activation(q_s[0:CFR, :], qps[0:CFR, 0:384], AF.Copy)
                    kps = pst([128, 512], "aux", 2)
                    nc.tensor.matmul(kps[0:CFR, 0:384], cqk_stk[0:CTOK, 0:CFR],
                                     qs[0:CTOK, E + half * 384: E + half * 384 + 384],
                                     start=True, stop=True)
                    prod = atile([CFR, 384], "prod", 2)
                    V.tensor_tensor(prod[0:CFR, :], kps[0:CFR, 0:384], q_s[0:CFR, :],
                                    ALU.mult)
                    with nc.allow_low_precision(reason="f32r logits partials"):
                        V.tensor_reduce(lp[0:CFR, half * 4: half * 4 + 4],
                                        prod[0:CFR, :].rearrange("p (h d) -> p h d", h=4),
                                        axis=AX.X, op=ALU.add)
                # logits [8, 52] = lp^T @ ssum (adds real+imag rows)
                lg_ps = pst([8, 128], "t", 2)
                nc.tensor.matmul(lg_ps[0:8, 0:52], lp[0:CFR, 0:8], ssum[0:CFR, 0:52],
                                 start=True, stop=True)
                smx = atile([8, 64], "smx", 2)
                V.tensor_copy(smx[0:8, 0:52], lg_ps[0:8, 0:52])
                mx = atile([8, 4], "mx", 2)
                V.tensor_reduce(mx[0:8, 0:4],
                                smx[0:8, 0:52].rearrange("p (b f) -> p b f", f=13),
                                axis=AX.X, op=ALU.max)
                V.tensor_tensor(smx[0:8, 0:52].rearrange("p (b f) -> p b f", f=13),
                                smx[0:8, 0:52].rearrange("p (b f) -> p b f", f=13),
                                mx[0:8, 0:4].unsqueeze(2).to_broadcast([8, 4, 13]),
                                ALU.subtract)
                sme = atile([8, 64], "sme", 2)
                S.activation(sme[0:8, 0:52], smx[0:8, 0:52], AF.Exp)
                sm_sum = atile([8, 4], "sm_sum", 2)
                V.tensor_reduce(sm_sum[0:8, 0:4],
                                sme[0:8, 0:52].rearrange("p (b f) -> p b f", f=13),
                                axis=AX.X, op=ALU.add)
                sm_rec = atile([8, 4], "sm_rec", 2)
                V.reciprocal(sm_rec[0:8, 0:4], sm_sum[0:8, 0:4])
                attrow = atile([8, CFR], "attrow", 2, FP32R)
                V.tensor_tensor(attrow[0:8, 0:52].rearrange("p (b f) -> p b f", f=13),
                                sme[0:8, 0:52].rearrange("p (b f) -> p b f", f=13),
                                sm_rec[0:8, 0:4].unsqueeze(2).to_broadcast([8, 4, 13]),
                                ALU.mult)
                V.tensor_copy(attrow[0:8, 52:104], attrow[0:8, 0:52])
                return attrow

            def emit_mid_back(c, qs, attrow):
                coff = c * CTOK
                ad_ps = pst([128, 128], "t", 2)
                nc.tensor.transpose(ad_ps[0:CFR, 0:8], attrow[0:8, 0:CFR].bitcast(FP32),
                                    eye[0:8, 0:8])
                att_s = atile([CFR, 8], "att_s", 2)
                V.tensor_copy(att_s[0:CFR, 0:8], ad_ps[0:CFR, 0:8])

                # v DFT + spectral filter; iDFT fused with transpose to c-major
                vsl = []
                for half in range(2):
                    vps = pst([128, 512], "aux", 2)
                    nc.tensor.matmul(vps[0:CFR, 0:384], cv_stk[0:CTOK, 0:CFR],
                                     qs[0:CTOK, 2 * E + half * 384: 2 * E + half * 384 + 384],
                                     start=True, stop=True)
                    vs = atile([CFR, 384], f"vs{half}", 2, FP32R)
                    V.tensor_tensor(vs[0:CFR, :].rearrange("p (h d) -> p h d", h=4),
                                    vps[0:CFR, 0:384].rearrange("p (h d) -> p h d", h=4),
                                    att_s[0:CFR, half * 4: half * 4 + 4]
                                    .unsqueeze(2).to_broadcast([CFR, 4, DH]),
                                    ALU.mult)
                    vsl.append(vs)
                for co in range(NCH):
                    o_ps = pst([128, 128], "t", 2)
                    nc.tensor.matmul(o_ps[0:128, 0:CTOK],
                                     vsl[co // 3][0:CFR, (co % 3) * 128:(co % 3) * 128 + 128],
                                     d_stk[0:CFR, 0:CTOK], start=True, stop=True)
                    S.activation(zb[:, co * WZ + coff: co * WZ + coff + CTOK],
                                 o_ps[:, 0:CTOK], AF.Copy)

            qs_l, ar_l = {}, {}
            for c in range(NCHUNK):
                qs_l[c] = emit_qkv(c)
                if c >= 1:
                    ar_l[c - 1] = emit_mid_front(c - 1, qs_l[c - 1])
                if c >= 2:
                    emit_mid_back(c - 2, qs_l[c - 2], ar_l[c - 2])
                    del qs_l[c - 2]
            ar_l[NCHUNK - 1] = emit_mid_front(NCHUNK - 1, qs_l[NCHUNK - 1])
            emit_mid_back(NCHUNK - 2, qs_l[NCHUNK - 2], ar_l[NCHUNK - 2])
            emit_mid_back(NCHUNK - 1, qs_l[NCHUNK - 1], ar_l[NCHUNK - 1])

        # ---- Wo GEMM + residual; LN2 stats + scaled zb per tile ----
        with tc.tile_pool(name=f"wo{l}", bufs=1) as wop:
            wo = []
            for ci in range(NCH):
                w = wop.tile([128, E], FP32R, tag=f"c{ci}", bufs=1, name=f"wo{l}_{ci}")
                nc.sync.dma_start(w[:, :], d["wo"][l, ci * 128:(ci + 1) * 128, :].bitcast(FP32R))
                wo.append(w)
            rows_m = wop.tile([1, TOKS], FP32, tag="rows_m", bufs=1, name="rows_m")
            rows_q = wop.tile([1, TOKS], FP32, tag="rows_q", bufs=1, name="rows_q")
            rrow = wop.tile([1, TOKS], FP32R, tag="rrow", bufs=1, name="rrow")

            def emit_wo_tile(off, n):
                for co in range(NCH):
                    p = pst([128, 512], "rot", 2)
                    for ci in range(NCH):
                        nc.tensor.matmul(p[:, 0:n], wo[ci][:, co * 128:(co + 1) * 128],
                                         zb[:, ci * WZ + off: ci * WZ + off + n],
                                         start=(ci == 0), stop=(ci == NCH - 1))
                    V.affine_then_add(h[:, co * WH + off: co * WH + off + n], p[:, 0:n],
                                      h[:, co * WH + off: co * WH + off + n],
                                      1.0, pp["bo_pp"][:, l * NCH + co: l * NCH + co + 1])

            def emit_ln2_chain(off, n):
                emit_rowmath(wop, rows_m, rows_q, rrow, off, n)
                rps = pst([128, 512], "aux", 2)
                nc.tensor.matmul(rps[:, 0:n], ones_row[0:1, 0:128],
                                 rrow[0:1, off:off + n], start=True, stop=True)
                for ci in range(NCH):
                    V.tensor_tensor(zb[:, ci * WZ + off: ci * WZ + off + n],
                                    h[:, ci * WH + off: ci * WH + off + n],
                                    rps[:, 0:n], ALU.mult)

            # staggered: Wo tiles in order [3,0,1,2]; stats trail by one tile,
            # chains by two, so the PE never queues behind scalar/vector links
            # and FFN tile 0's zb is ready early.
            order = [TT4[3], TT4[0], TT4[1], TT4[2]]
            for i, (off, n) in enumerate(order):
                emit_wo_tile(off, n)
                if i >= 1:
                    emit_stats(rows_m, rows_q, h, WH, order[i - 1][0], order[i - 1][1])
                if i >= 2:
                    emit_ln2_chain(order[i - 2][0], order[i - 2][1])
            emit_stats(rows_m, rows_q, h, WH, order[3][0], order[3][1])
            emit_ln2_chain(order[2][0], order[2][1])
            emit_ln2_chain(order[3][0], order[3][1])

        # ---- FFN: th-pair streamed weights, f2 accumulated in PSUM ----
        F2TAG = {0: "aux", 1: "aux", 2: "bi", 3: "bi", 4: "t", 5: "t"}
        with tc.tile_pool(name=f"ff{l}", bufs=1) as ffp:
            def load_th(th):
                w1, w2 = [], []
                for ci in range(NCH):
                    w = ffp.tile([128, 512], FP32R, tag=f"w1_{ci}", bufs=3, name=f"wf1_{ci}")
                    nc.sync.dma_start(w[:, :], d["wf1"][l, ci * 128:(ci + 1) * 128,
                                                        th * 512:(th + 1) * 512].bitcast(FP32R))
                    w1.append(w)
                for ci4 in range(4):
                    w = ffp.tile([128, E], FP32R, tag=f"w2_{ci4}", bufs=3, name=f"wf2_{ci4}")
                    nc.sync.dma_start(w[:, :], d["wf2"][l, th * 512 + ci4 * 128:
                                                        th * 512 + (ci4 + 1) * 128, :].bitcast(FP32R))
                    w2.append(w)
                return w1, w2

            for pr in range(3):
                wpair = [load_th(2 * pr), load_th(2 * pr + 1)]
                for (off, n) in TT4:
                    gth = []
                    for thl in range(2):
                        th = 2 * pr + thl
                        g_ = ffp.tile([128, 4 * 400], FP32R, tag=f"gth{thl}", bufs=2, name="gth")
                        for co4 in range(4):
                            p = pst([128, 512], "rot", 2)
                            for ci in range(NCH):
                                nc.tensor.matmul(p[:, 0:n],
                                                 wpair[thl][0][ci][:, co4 * 128:(co4 + 1) * 128],
                                                 zb[:, ci * WZ + off: ci * WZ + off + n],
                                                 start=(ci == 0), stop=(ci == NCH - 1))
                            S.activation(g_[:, co4 * 400: co4 * 400 + n], p[:, 0:n], AF.Gelu,
                                         bias=pp["bf1_pp"][:, l * 24 + th * 4 + co4:
                                                           l * 24 + th * 4 + co4 + 1])
                        gth.append(g_)
                    for co in range(NCH):
                        p2 = pst([128, 512], F2TAG[co], 2)
                        for thl in range(2):
                            for ci4 in range(4):
                                nc.tensor.matmul(p2[:, 0:n],
                                                 wpair[thl][1][ci4][:, co * 128:(co + 1) * 128],
                                                 gth[thl][:, ci4 * 400: ci4 * 400 + n],
                                                 start=(thl == 0 and ci4 == 0),
                                                 stop=(thl == 1 and ci4 == 3))
                        hs = h[:, co * WH + off: co * WH + off + n]
                        if pr == 0:
                            V.affine_then_add(hs, p2[:, 0:n], hs, 1.0,
                                              pp["bf2_pp"][:, l * NCH + co: l * NCH + co + 1])
                        else:
                            V.tensor_tensor(hs, hs, p2[:, 0:n], ALU.add)

    # =======================================================================
    # SSM conv (psum-accumulated taps) + final LN + fused head
    # =======================================================================
    with tc.tile_pool(name="ssm", bufs=1) as sp:
        wout_sb = sp.tile([128, NCH * NCLS], FP32R, tag="wout", bufs=1, name="wout_sb")
        nc.sync.dma_start(wout_sb[:, :].rearrange("p (c o) -> p c o", o=NCLS),
                          d["wout"][:, :].rearrange("(c p) o -> p c o", p=128).bitcast(FP32R))

        # build padded copy of h in zb
        for ci in range(NCH):
            zv = zb[:, ci * WZ: ci * WZ + PADW]
            V.memset(zv[:, 0:1].bitcast(FP32), 0.0)
            V.memset(zv[:, 1:].rearrange("p (b f) -> p b f", f=28)[:, :, 25:28].bitcast(FP32), 0.0)
            V.tensor_copy(zv[:, 1:].rearrange("p (b f) -> p b f", f=28)[:, :, 0:25],
                          h[:, ci * WH: ci * WH + WH].rearrange("p (b f) -> p b f", f=25))

        # ssm conv: h += conv(h_pad) + ssm_b  (one tap at a time, weights
        # streamed); the final fused LN + head runs per tile inside tap 2.
        rows_m = sp.tile([1, TOKS], FP32, tag="rows_m", bufs=1, name="rows_m3")
        rows_q = sp.tile([1, TOKS], FP32, tag="rows_q", bufs=1, name="rows_q3")
        rrow = sp.tile([1, TOKS], FP32R, tag="rrow", bufs=1, name="rrow3")
        hm = ttile([NCLS, BL], "hm", 1)
        def emit_conv_tile(tap, sh_, sw, boff, nb):
            off, n = boff * 25, nb * 25
            crhs = []
            for ci in range(NCH):
                ct = sp.tile([128, 512], FP32R, tag="crhs", bufs=8, name="crhs")
                S.activation(ct[:, 0:n].rearrange("p (b f) -> p b f", f=25),
                             zb[:, ci * WZ + 1 + sh_ + boff * 28:][0:128, 0:nb * 28]
                             .rearrange("p (b f) -> p b f", f=28)[:, :, 0:25].bitcast(FP32),
                             AF.Copy)
                crhs.append(ct)
            for co in range(NCH):
                p = pst([128, 512], "rot", 2)
                for ci in range(NCH):
                    nc.tensor.matmul(p[:, 0:n], sw[ci][:, co * 128:(co + 1) * 128],
                                     crhs[ci][:, 0:n],
                                     start=(ci == 0), stop=(ci == NCH - 1))
                hs = h[:, co * WH + off: co * WH + off + n]
                if tap == 0:
                    V.affine_then_add(hs, p[:, 0:n], hs, 1.0, pp["ssmb_pp"][:, co:co + 1])
                else:
                    V.tensor_tensor(hs, hs, p[:, 0:n], ALU.add)

        def emit_ln3_chain(boff, nb):
            # scaled activations land at pad-aligned offsets (28*boff): that
            # region of the h_pad is dead once this tile's tap-2 conv ran,
            # while later tiles' pad columns stay untouched.
            off, n, zoff = boff * 25, nb * 25, boff * 28
            emit_rowmath(sp, rows_m, rows_q, rrow, off, n)
            rps = pst([128, 512], "aux", 2)
            nc.tensor.matmul(rps[:, 0:n], ones_row[0:1, 0:128],
                             rrow[0:1, off:off + n], start=True, stop=True)
            for ci in range(NCH):
                V.tensor_tensor(zb[:, ci * WZ + zoff: ci * WZ + zoff + n],
                                h[:, ci * WH + off: ci * WH + off + n],
                                rps[:, 0:n], ALU.mult)
            hp = pst([128, 512], "bi", 2)
            for ci in range(NCH):
                nc.tensor.matmul(hp[0:NCLS, 0:n], wout_sb[:, ci * NCLS:(ci + 1) * NCLS],
                                 zb[:, ci * WZ + zoff: ci * WZ + zoff + n],
                                 start=(ci == 0), stop=(ci == NCH - 1))
            V.tensor_reduce(hm[0:NCLS, boff:boff + nb],
                            hp[0:NCLS, 0:n].rearrange("p (b f) -> p b f", f=25),
                            axis=AX.X, op=ALU.add)
            outT = ttile([NCLS, 16], "outT", 2)
            V.tensor_scalar(outT[0:NCLS, 0:nb], hm[0:NCLS, boff:boff + nb],
                            bout_pp[0:NCLS, 0:1], None, ALU.add)
            otp = pst([128, 128], "t", 2)
            nc.tensor.transpose(otp[0:nb, 0:NCLS], outT[0:NCLS, 0:nb], eye[0:NCLS, 0:NCLS])
            ofin = ttile([16, NCLS], "ofin", 2)
            V.tensor_copy(ofin[0:nb, 0:NCLS], otp[0:nb, 0:NCLS])
            nc.sync.dma_start(out_d[boff:boff + nb, :], ofin[0:nb, 0:NCLS])

        for tap, sh_ in ((0, -1), (1, 0), (2, 1)):
            sw = []
            for ci in range(NCH):
                w = sp.tile([128, E], FP32R, tag=f"swc{ci}", bufs=2, name=f"ssw{ci}")
                nc.sync.dma_start(w[:, :], d["ssmw"][tap, ci * 128:(ci + 1) * 128, :].bitcast(FP32R))
                sw.append(w)
            if tap < 2:
                for (boff, nb) in BT4:
                    emit_conv_tile(tap, sh_, sw, boff, nb)
            else:
                # staggered: conv tiles [3,0,1,2]; final-LN stats trail by one
                # tile, the chain (rstd + head + output) by two.
                order = [BT4[3], BT4[0], BT4[1], BT4[2]]
                for i, (boff, nb) in enumerate(order):
                    emit_conv_tile(tap, sh_, sw, boff, nb)
                    if i >= 1:
                        emit_stats(rows_m, rows_q, h, WH, order[i - 1][0] * 25,
                                   order[i - 1][1] * 25)
                    if i >= 2:
                        emit_ln3_chain(order[i - 2][0], order[i - 2][1])
                emit_stats(rows_m, rows_q, h, WH, order[3][0] * 25, order[3][1] * 25)
                emit_ln3_chain(order[2][0], order[2][1])
                emit_ln3_chain(order[3][0], order[3][1])


# ---------------------------------------------------------------------------
_NC = None


def _get_nc():
    global _NC
    if _NC is None:
        _NC = _build()
    return _NC


def _run(inputs, trace=False):
    nc = _get_nc()
    sh, xs = _prep(inputs)
    in_maps = [dict(sh, xcm=xs[i]) for i in range(NCORES)]
    res = run_bass_kernel_spmd(nc, in_maps, core_ids=list(range(NCORES)), trace=trace)
    out = np.concatenate([res.results[i]["out"] for i in range(NCORES)], axis=0)
    return out.astype(np.float32), res


def kernel(**inputs):
    out, _ = _run(inputs, trace=False)
    return out
